# revision 3
# baseline (speedup 1.0000x reference)
"""Confusion-matrix kernel v5 for Trainium2 (8 NeuronCores, data-parallel over batch).

Per batch b (one per core):
    pred[n]  = argmax_c input[b, c, n]            (n = pixel, N = H*W)
    cm[i, j] = sum_n target[b, i, n] * (pred[n] == j)
    rs[i]    = sum_n target[b, i, n]
Host: cm_b = cm / (rs + 1e-8); out = mean_b cm_b.

CLASS-MAJOR layout: per group (128 partitions x 6 pixel slots = 768 pixels),
x is stored as 21 class-blocks of 6 slot values -> 126 fp16 cols/group (no
pad column). The per-pixel max runs as a (7,7,4,2,1) block-overlap tree in
2x-mode tensor_tensor ops (63 cyc/group):
    T1: tb1 = max(x[b0:b7],   x[b7:b14])
    T2: tb2 = max(tb1,        x[b14:b21])
    T3: t4  = max(tb2[b0:b4], tb2[b3:b7])
    T4: t2  = max(t4[b0:b2],  t4[b2:b4])
    T5: m1  = max(t2[b0],     t2[b1])        -> per-slot max, 6 cols
    h[g, j, s] = is_ge(x[g, j, s], m1[g, s])  (63 cyc/group; bcast keeps 2x)
h is 132 cols/group: 126 one-hot + 6 ones cols (rs via matmul). The is_ge of
tile t-1 is woven between T2(t) and T3(t) so the dependent-chain DRAIN of
T2 overlaps an independent op.

One matmul per group: lhsT = y block [128, 128] fp8e4 (126 used),
rhs = h block [128, 132] fp16, accumulated into one [128, 132] f32 PSUM
tile; ACT does the final PSUM->SBUF copy. Host extracts
cm[i,j] = sum_s out[i*6+s, j*6+s], rs[i] = sum_s out[i*6+s, 126+s].

Engines: SP x-loads + out-store | ACT y-loads + final copy | GPSIMD h
ones-init | DVE tree + is_ge | PE matmuls.
"""

from contextlib import ExitStack

import ml_dtypes
import numpy as np

import concourse.bass as bass
import concourse.mybir as mybir
from concourse.bass_utils import run_bass_kernel_spmd

B, C, H, W = 8, 21, 512, 512
N = H * W              # 262144 pixels per batch
P = 128                # SBUF partitions
S = 6                  # pixel slots per PE row
CW = C * S             # x cols per group (126), class-major
YW = 128               # y group width: 126 + 2 pad cols
HWC = (C + 1) * S      # h cols per group (132): 126 one-hot + 6 ones
NG = 342               # total groups per core (342*768 = 262656 >= N)
NPAD = NG * P * S      # padded pixel count
G_TILES = [10, 22, 44, 57, 57, 57, 57, 26, 12]
NT = len(G_TILES)
G_OFF = [sum(G_TILES[:i]) for i in range(NT)]
GMAX = max(G_TILES)
NEG = -65504.0
N_CORES = 8

_CACHED_NC = None


def build_nc():
    nc = bass.Bass()
    x = nc.declare_dram_parameter("x", [P, NG * CW], mybir.dt.float16, isOutput=False)
    y = nc.declare_dram_parameter("y", [P, NG * YW], mybir.dt.float8e4, isOutput=False)
    out = nc.declare_dram_parameter("out", [P, HWC], mybir.dt.float32, isOutput=True)

    mx = mybir.AluOpType.max

    with ExitStack() as ctx:
        xs = [
            ctx.enter_context(
                nc.sbuf_tensor(f"xsb{i}", [P, GMAX * CW], mybir.dt.float16)
            )
            for i in range(3)
        ]
        ys = [
            ctx.enter_context(
                nc.sbuf_tensor(f"ysb{i}", [P, GMAX * YW], mybir.dt.float8e4)
            )
            for i in range(2)
        ]
        hs = [
            ctx.enter_context(
                nc.sbuf_tensor(f"hsb{i}", [P, GMAX * HWC], mybir.dt.float16)
            )
            for i in range(2)
        ]
        tb1 = ctx.enter_context(nc.sbuf_tensor("tb1b", [P, GMAX * 42], mybir.dt.float16))
        tb2 = ctx.enter_context(nc.sbuf_tensor("tb2b", [P, GMAX * 42], mybir.dt.float16))
        t4 = ctx.enter_context(nc.sbuf_tensor("t4b", [P, GMAX * 24], mybir.dt.float16))
        t2 = ctx.enter_context(nc.sbuf_tensor("t2b", [P, GMAX * 12], mybir.dt.float16))
        m1s = [
            ctx.enter_context(nc.sbuf_tensor(f"m1b{i}", [P, GMAX * 6], mybir.dt.float16))
            for i in range(2)
        ]
        osb = ctx.enter_context(nc.sbuf_tensor("osb", [P, HWC], mybir.dt.float32))
        cm_psum = ctx.enter_context(nc.psum_tensor("cmps", [P, HWC], mybir.dt.float32))

        block = ctx.enter_context(nc.Block())
        sxs = [ctx.enter_context(nc.semaphore(f"sx{i}")) for i in range(3)]
        sys_ = [ctx.enter_context(nc.semaphore(f"sy{i}")) for i in range(2)]
        shd = ctx.enter_context(nc.semaphore("shd"))    # DVE isge(t) done, = t+1
        si = ctx.enter_context(nc.semaphore("si"))      # gpsimd ones init done
        sp = ctx.enter_context(nc.semaphore("sp"))      # PE tile matmuls done, = t+1
        sv2 = ctx.enter_context(nc.semaphore("sv2"))    # final psum copy done
        so = ctx.enter_context(nc.semaphore("so"))      # out DMA done

        def mview(buf, w, t):
            return (
                buf[:]
                .rearrange("p (g c) -> p g c", c=w)[:, 0 : G_TILES[t], :]
            )

        @block.sync
        def _(sync):
            for t in range(NT):
                if t >= 3:
                    sync.wait_ge(shd, t - 2)  # isge(t-3) freed x slot
                cols = G_TILES[t] * CW
                sync.dma_start(
                    out=xs[t % 3][:, 0:cols],
                    in_=x[:, G_OFF[t] * CW : G_OFF[t] * CW + cols],
                ).then_inc(sxs[t % 3], 16)
            sync.wait_ge(sv2, 1)
            sync.dma_start(out=out[:], in_=osb[:]).then_inc(so, 16)
            sync.wait_ge(so, 16)

        @block.scalar
        def _(scalar):
            # defer the first y-loads until x0 has landed: x feeds the DVE
            # ramp (the critical path).
            scalar.wait_ge(sxs[0], 16)
            for t in range(NT):
                if t >= 2:
                    scalar.wait_ge(sp, t - 1)  # matmul(t-2) freed y slot
                cols = G_TILES[t] * YW
                scalar.dma_start(
                    out=ys[t % 2][:, 0:cols],
                    in_=y[:, G_OFF[t] * YW : G_OFF[t] * YW + cols],
                ).then_inc(sys_[t % 2], 16)
            scalar.wait_ge(sp, NT)
            nc.scalar.copy(out=osb[:], in_=cm_psum[:]).then_inc(sv2, 1)

        @block.gpsimd
        def _(gpsimd):
            h30 = hs[0][:].rearrange("p (g w) -> p g w", w=HWC)
            h31 = hs[1][:].rearrange("p (g w) -> p g w", w=HWC)
            nc.gpsimd.memset(h30[:, :, CW:HWC], 1.0)
            nc.gpsimd.memset(h31[:, :, CW:HWC], 1.0).then_inc(si, 1)

        @block.vector
        def _(vector):
            def isge(t):
                # h(t) one-hot: compare x against per-slot max, broadcast
                # over the 21 class blocks (inner step 1 keeps 2x mode)
                if t >= 2:
                    vector.wait_ge(sp, t - 1)  # matmul(t-2) freed h slot
                G = G_TILES[t]
                x4 = (
                    xs[t % 3][:]
                    .rearrange("p (g j s) -> p g j s", j=C, s=S)[:, 0:G, :, :]
                )
                h4 = (
                    hs[t % 2][:]
                    .rearrange("p (g j s) -> p g j s", j=C + 1, s=S)
                )
                m1b = (
                    m1s[t % 2][:]
                    .rearrange("p (g s) -> p g s", s=S)[:, 0:G, :]
                    .unsqueeze(2)
                    .to_broadcast((P, G, C, S))
                )
                nc.vector.tensor_tensor(
                    out=h4[:, 0:G, 0:C, :],
                    in0=x4,
                    in1=m1b,
                    op=mybir.AluOpType.is_ge,
                ).then_inc(shd, 1)

            for t in range(NT):
                vector.wait_ge(sxs[t % 3], 16 * (t // 3 + 1))
                x3 = mview(xs[t % 3], CW, t)
                tb1v = mview(tb1, 42, t)
                tb2v = mview(tb2, 42, t)
                t4v = mview(t4, 24, t)
                t2v = mview(t2, 12, t)
                m1v = mview(m1s[t % 2], 6, t)
                nc.vector.tensor_tensor(
                    out=tb1v, in0=x3[:, :, 0:42], in1=x3[:, :, 42:84], op=mx
                )
                nc.vector.tensor_tensor(
                    out=tb2v, in0=tb1v, in1=x3[:, :, 84:126], op=mx
                )
                if t >= 1:
                    # weave isge(t-1): independent of T2(t), overlaps its DRAIN
                    isge(t - 1)
                nc.vector.tensor_tensor(
                    out=t4v, in0=tb2v[:, :, 0:24], in1=tb2v[:, :, 18:42], op=mx
                )
                nc.vector.tensor_tensor(
                    out=t2v, in0=t4v[:, :, 0:12], in1=t4v[:, :, 12:24], op=mx
                )
                nc.vector.tensor_tensor(
                    out=m1v, in0=t2v[:, :, 0:6], in1=t2v[:, :, 6:12], op=mx
                )
            isge(NT - 1)

        @block.tensor
        def _(tensor):
            tensor.wait_ge(si, 1)
            for t in range(NT):
                tensor.wait_ge(sys_[t % 2], 16 * (t // 2 + 1))
                tensor.wait_ge(shd, t + 1)
                for g in range(G_TILES[t]):
                    mm = nc.tensor.matmul(
                        out=cm_psum[:],
                        lhsT=ys[t % 2][:, g * YW : (g + 1) * YW],
                        rhs=hs[t % 2][:, g * HWC : (g + 1) * HWC],
                        start=(t == 0 and g == 0),
                        stop=(t == NT - 1 and g == G_TILES[t] - 1),
                    )
                mm.then_inc(sp, 1)

    return nc


def _get_nc():
    global _CACHED_NC
    if _CACHED_NC is None:
        _CACHED_NC = build_nc()
    return _CACHED_NC


def make_in_maps(input, target):
    inp = np.asarray(input, dtype=np.float32)
    tgt = np.asarray(target, dtype=np.float32)
    in_maps = []
    for b in range(B):
        xb = inp[b].reshape(C, N).T  # [N, C]
        xq = np.full((NPAD, C), NEG, dtype=np.float16)
        xq[:N] = xb
        # pad pixels keep x = NEG everywhere -> h row all-ones but y rows are 0
        # class-major: [P, NG, C, S]
        x_dev = np.ascontiguousarray(
            xq.reshape(NG, S, P, C).transpose(2, 0, 3, 1)
        ).reshape(P, NG * CW)

        yb = tgt[b].reshape(C, N).T  # [N, C]
        yq = np.zeros((NPAD, C), dtype=np.float32)
        yq[:N] = yb
        y4 = yq.reshape(NG, S, P, C).transpose(2, 0, 3, 1)  # [P,NG,C,S]
        y_dev = np.zeros((P, NG, YW), dtype=ml_dtypes.float8_e4m3)
        y_dev[..., :CW] = y4.reshape(P, NG, CW).astype(ml_dtypes.float8_e4m3)
        in_maps.append({"x": x_dev, "y": y_dev.reshape(P, NG * YW)})
    return in_maps


def postprocess(outs):
    final = np.zeros((C, C), dtype=np.float64)
    for o in outs:
        o = np.asarray(o, dtype=np.float64)  # [128, 132]
        ov = o[:CW, :CW].reshape(C, S, C, S)
        cm = np.einsum("isjs->ij", ov)
        rsv = o[:CW, CW:HWC].reshape(C, S, S)
        rs = np.einsum("iss->i", rsv).reshape(C, 1)
        final += cm / (rs + 1e-8)
    return (final / len(outs)).astype(np.float32)


def kernel(input, target):
    nc = _get_nc()
    in_maps = make_in_maps(input, target)
    res = run_bass_kernel_spmd(nc, in_maps, list(range(N_CORES)))
    return postprocess([r["out"] for r in res.results])


# revision 5
# speedup vs baseline: 1.0945x; 1.0945x over previous
"""Confusion-matrix kernel v5 for Trainium2 (8 NeuronCores, data-parallel over batch).

Per batch b (one per core):
    pred[n]  = argmax_c input[b, c, n]            (n = pixel, N = H*W)
    cm[i, j] = sum_n target[b, i, n] * (pred[n] == j)
    rs[i]    = sum_n target[b, i, n]
Host: cm_b = cm / (rs + 1e-8); out = mean_b cm_b.

CLASS-MAJOR layout: per group (128 partitions x 6 pixel slots = 768 pixels),
x is stored as 21 class-blocks of 6 slot values -> 126 fp16 cols/group (no
pad column). The per-pixel max runs as a (7,7,4,2,1) block-overlap tree in
2x-mode tensor_tensor ops (63 cyc/group):
    T1: tb1 = max(x[b0:b7],   x[b7:b14])
    T2: tb2 = max(tb1,        x[b14:b21])
    T3: t4  = max(tb2[b0:b4], tb2[b3:b7])
    T4: t2  = max(t4[b0:b2],  t4[b2:b4])
    T5: m1  = max(t2[b0],     t2[b1])        -> per-slot max, 6 cols
    h[g, j, s] = is_ge(x[g, j, s], m1[g, s])  (63 cyc/group; bcast keeps 2x)
h is 132 cols/group: 126 one-hot + 6 ones cols (rs via matmul). The is_ge of
tile t-1 is woven between T2(t) and T3(t) so the dependent-chain DRAIN of
T2 overlaps an independent op.

One matmul per group: lhsT = y block [128, 128] fp8e4 (126 used),
rhs = h block [128, 132] fp16, accumulated into one [128, 132] f32 PSUM
tile; ACT does the final PSUM->SBUF copy. Host extracts
cm[i,j] = sum_s out[i*6+s, j*6+s], rs[i] = sum_s out[i*6+s, 126+s].

Engines: SP x-loads + out-store | ACT y-loads + final copy | GPSIMD h
ones-init | DVE tree + is_ge | PE matmuls.
"""

from contextlib import ExitStack

import ml_dtypes
import numpy as np

import concourse.bass as bass
import concourse.mybir as mybir
from concourse.bass_utils import run_bass_kernel_spmd

B, C, H, W = 8, 21, 512, 512
N = H * W              # 262144 pixels per batch
P = 128                # SBUF partitions
S = 6                  # pixel slots per PE row
CW = C * S             # x cols per group (126), class-major
YW = 128               # y group width: 126 + 2 pad cols
HWC = (C + 1) * S      # h cols per group (132): 126 one-hot + 6 ones
NG = 342               # total groups per core (342*768 = 262656 >= N)
NPAD = NG * P * S      # padded pixel count
G_TILES = [10, 22, 44, 57, 57, 57, 57, 26, 12]
NT = len(G_TILES)
G_OFF = [sum(G_TILES[:i]) for i in range(NT)]
GMAX = max(G_TILES)
NEG = -65504.0
N_CORES = 8

_CACHED_NC = None


def build_nc():
    nc = bass.Bass()
    x = nc.declare_dram_parameter("x", [P, NG * CW], mybir.dt.float16, isOutput=False)
    y = nc.declare_dram_parameter("y", [P, NG * YW], mybir.dt.float8e4, isOutput=False)
    out = nc.declare_dram_parameter("out", [P, HWC], mybir.dt.float32, isOutput=True)

    mx = mybir.AluOpType.max

    with ExitStack() as ctx:
        xs = [
            ctx.enter_context(
                nc.sbuf_tensor(f"xsb{i}", [P, GMAX * CW], mybir.dt.float16)
            )
            for i in range(4)
        ]
        ys = [
            ctx.enter_context(
                nc.sbuf_tensor(f"ysb{i}", [P, GMAX * YW], mybir.dt.float8e4)
            )
            for i in range(3)
        ]
        hs = [
            ctx.enter_context(
                nc.sbuf_tensor(f"hsb{i}", [P, GMAX * HWC], mybir.dt.float16)
            )
            for i in range(3)
        ]
        tb1 = ctx.enter_context(nc.sbuf_tensor("tb1b", [P, GMAX * 42], mybir.dt.float16))
        tb2 = ctx.enter_context(nc.sbuf_tensor("tb2b", [P, GMAX * 42], mybir.dt.float16))
        t4 = ctx.enter_context(nc.sbuf_tensor("t4b", [P, GMAX * 24], mybir.dt.float16))
        t2 = ctx.enter_context(nc.sbuf_tensor("t2b", [P, GMAX * 12], mybir.dt.float16))
        m1s = [
            ctx.enter_context(nc.sbuf_tensor(f"m1b{i}", [P, GMAX * 6], mybir.dt.float16))
            for i in range(2)
        ]
        osb = ctx.enter_context(nc.sbuf_tensor("osb", [P, HWC], mybir.dt.float32))
        cm_psum = ctx.enter_context(nc.psum_tensor("cmps", [P, HWC], mybir.dt.float32))

        block = ctx.enter_context(nc.Block())
        sxs = [ctx.enter_context(nc.semaphore(f"sx{i}")) for i in range(4)]
        sys_ = [ctx.enter_context(nc.semaphore(f"sy{i}")) for i in range(3)]
        shd = ctx.enter_context(nc.semaphore("shd"))    # DVE isge(t) done, = t+1
        si = ctx.enter_context(nc.semaphore("si"))      # gpsimd ones init done
        sp = ctx.enter_context(nc.semaphore("sp"))      # PE tile matmuls done, = t+1
        sv2 = ctx.enter_context(nc.semaphore("sv2"))    # final psum copy done
        so = ctx.enter_context(nc.semaphore("so"))      # out DMA done

        def mview(buf, w, t):
            return (
                buf[:]
                .rearrange("p (g c) -> p g c", c=w)[:, 0 : G_TILES[t], :]
            )

        @block.sync
        def _(sync):
            for t in range(NT):
                if t >= 4:
                    sync.wait_ge(shd, t - 3)  # isge(t-4) freed x slot
                cols = G_TILES[t] * CW
                sync.dma_start(
                    out=xs[t % 4][:, 0:cols],
                    in_=x[:, G_OFF[t] * CW : G_OFF[t] * CW + cols],
                ).then_inc(sxs[t % 4], 16)
            sync.wait_ge(sv2, 1)
            sync.dma_start(out=out[:], in_=osb[:]).then_inc(so, 16)
            sync.wait_ge(so, 16)

        @block.scalar
        def _(scalar):
            # defer the first y-loads until x0 has landed: x feeds the DVE
            # ramp (the critical path).
            scalar.wait_ge(sxs[0], 16)
            for t in range(NT):
                if t >= 3:
                    scalar.wait_ge(sp, t - 2)  # matmul(t-3) freed y slot
                cols = G_TILES[t] * YW
                scalar.dma_start(
                    out=ys[t % 3][:, 0:cols],
                    in_=y[:, G_OFF[t] * YW : G_OFF[t] * YW + cols],
                ).then_inc(sys_[t % 3], 16)


        @block.gpsimd
        def _(gpsimd):
            h3v = [h[:].rearrange("p (g w) -> p g w", w=HWC) for h in hs]
            nc.gpsimd.memset(h3v[0][:, :, CW:HWC], 1.0)
            nc.gpsimd.memset(h3v[1][:, :, CW:HWC], 1.0)
            nc.gpsimd.memset(h3v[2][:, :, CW:HWC], 1.0).then_inc(si, 1)

        @block.vector
        def _(vector):
            def isge(t):
                # h(t) one-hot: compare x against per-slot max, broadcast
                # over the 21 class blocks (inner step 1 keeps 2x mode)
                if t >= 3:
                    vector.wait_ge(sp, t - 2)  # matmul(t-3) freed h slot
                G = G_TILES[t]
                x4 = (
                    xs[t % 4][:]
                    .rearrange("p (g j s) -> p g j s", j=C, s=S)[:, 0:G, :, :]
                )
                h4 = (
                    hs[t % 3][:]
                    .rearrange("p (g j s) -> p g j s", j=C + 1, s=S)
                )
                m1b = (
                    m1s[t % 2][:]
                    .rearrange("p (g s) -> p g s", s=S)[:, 0:G, :]
                    .unsqueeze(2)
                    .to_broadcast((P, G, C, S))
                )
                nc.vector.tensor_tensor(
                    out=h4[:, 0:G, 0:C, :],
                    in0=x4,
                    in1=m1b,
                    op=mybir.AluOpType.is_ge,
                ).then_inc(shd, 1)

            for t in range(NT):
                vector.wait_ge(sxs[t % 4], 16 * (t // 4 + 1))
                x3 = mview(xs[t % 4], CW, t)
                tb1v = mview(tb1, 42, t)
                tb2v = mview(tb2, 42, t)
                t4v = mview(t4, 24, t)
                t2v = mview(t2, 12, t)
                m1v = mview(m1s[t % 2], 6, t)
                nc.vector.tensor_tensor(
                    out=tb1v, in0=x3[:, :, 0:42], in1=x3[:, :, 42:84], op=mx
                )
                nc.vector.tensor_tensor(
                    out=tb2v, in0=tb1v, in1=x3[:, :, 84:126], op=mx
                )
                if t >= 1:
                    # weave isge(t-1): independent of T2(t), overlaps its DRAIN
                    isge(t - 1)
                nc.vector.tensor_tensor(
                    out=t4v, in0=tb2v[:, :, 0:24], in1=tb2v[:, :, 18:42], op=mx
                )
                nc.vector.tensor_tensor(
                    out=t2v, in0=t4v[:, :, 0:12], in1=t4v[:, :, 12:24], op=mx
                )
                nc.vector.tensor_tensor(
                    out=m1v, in0=t2v[:, :, 0:6], in1=t2v[:, :, 6:12], op=mx
                )
            isge(NT - 1)
            vector.wait_ge(sp, NT)
            nc.vector.tensor_copy(osb[:], cm_psum[:]).then_inc(sv2, 1)

        @block.tensor
        def _(tensor):
            tensor.wait_ge(si, 1)
            for t in range(NT):
                tensor.wait_ge(sys_[t % 3], 16 * (t // 3 + 1))
                tensor.wait_ge(shd, t + 1)
                for g in range(G_TILES[t]):
                    mm = nc.tensor.matmul(
                        out=cm_psum[:],
                        lhsT=ys[t % 3][:, g * YW : (g + 1) * YW],
                        rhs=hs[t % 3][:, g * HWC : (g + 1) * HWC],
                        start=(t == 0 and g == 0),
                        stop=(t == NT - 1 and g == G_TILES[t] - 1),
                    )
                mm.then_inc(sp, 1)

    return nc


def _get_nc():
    global _CACHED_NC
    if _CACHED_NC is None:
        _CACHED_NC = build_nc()
    return _CACHED_NC


def make_in_maps(input, target):
    inp = np.asarray(input, dtype=np.float32)
    tgt = np.asarray(target, dtype=np.float32)
    in_maps = []
    for b in range(B):
        xb = inp[b].reshape(C, N).T  # [N, C]
        xq = np.full((NPAD, C), NEG, dtype=np.float16)
        xq[:N] = xb
        # pad pixels keep x = NEG everywhere -> h row all-ones but y rows are 0
        # class-major: [P, NG, C, S]
        x_dev = np.ascontiguousarray(
            xq.reshape(NG, S, P, C).transpose(2, 0, 3, 1)
        ).reshape(P, NG * CW)

        yb = tgt[b].reshape(C, N).T  # [N, C]
        yq = np.zeros((NPAD, C), dtype=np.float32)
        yq[:N] = yb
        y4 = yq.reshape(NG, S, P, C).transpose(2, 0, 3, 1)  # [P,NG,C,S]
        y_dev = np.zeros((P, NG, YW), dtype=ml_dtypes.float8_e4m3)
        y_dev[..., :CW] = y4.reshape(P, NG, CW).astype(ml_dtypes.float8_e4m3)
        in_maps.append({"x": x_dev, "y": y_dev.reshape(P, NG * YW)})
    return in_maps


def postprocess(outs):
    final = np.zeros((C, C), dtype=np.float64)
    for o in outs:
        o = np.asarray(o, dtype=np.float64)  # [128, 132]
        ov = o[:CW, :CW].reshape(C, S, C, S)
        cm = np.einsum("isjs->ij", ov)
        rsv = o[:CW, CW:HWC].reshape(C, S, S)
        rs = np.einsum("iss->i", rsv).reshape(C, 1)
        final += cm / (rs + 1e-8)
    return (final / len(outs)).astype(np.float32)


def kernel(input, target):
    nc = _get_nc()
    in_maps = make_in_maps(input, target)
    res = run_bass_kernel_spmd(nc, in_maps, list(range(N_CORES)))
    return postprocess([r["out"] for r in res.results])


# revision 7
# speedup vs baseline: 1.1253x; 1.0281x over previous
"""Confusion-matrix kernel v5 for Trainium2 (8 NeuronCores, data-parallel over batch).

Per batch b (one per core):
    pred[n]  = argmax_c input[b, c, n]            (n = pixel, N = H*W)
    cm[i, j] = sum_n target[b, i, n] * (pred[n] == j)
    rs[i]    = sum_n target[b, i, n]
Host: cm_b = cm / (rs + 1e-8); out = mean_b cm_b.

CLASS-MAJOR layout: per group (128 partitions x 6 pixel slots = 768 pixels),
x is stored as 21 class-blocks of 6 slot values -> 126 fp16 cols/group (no
pad column). The per-pixel max runs as a (7,7,4,2,1) block-overlap tree in
2x-mode tensor_tensor ops (63 cyc/group):
    T1: tb1 = max(x[b0:b7],   x[b7:b14])
    T2: tb2 = max(tb1,        x[b14:b21])
    T3: t4  = max(tb2[b0:b4], tb2[b3:b7])
    T4: t2  = max(t4[b0:b2],  t4[b2:b4])
    T5: m1  = max(t2[b0],     t2[b1])        -> per-slot max, 6 cols
    h[g, j, s] = is_ge(x[g, j, s], m1[g, s])  (63 cyc/group; bcast keeps 2x)
h is 132 cols/group: 126 one-hot + 6 ones cols (rs via matmul). The is_ge of
tile t-1 is woven between T2(t) and T3(t) so the dependent-chain DRAIN of
T2 overlaps an independent op.

One matmul per group: lhsT = y block [128, 128] fp8e4 (126 used),
rhs = h block [128, 132] fp16, accumulated into one [128, 132] f32 PSUM
tile; ACT does the final PSUM->SBUF copy. Host extracts
cm[i,j] = sum_s out[i*6+s, j*6+s], rs[i] = sum_s out[i*6+s, 126+s].

Engines: SP x-loads + out-store | ACT y-loads + final copy | GPSIMD h
ones-init | DVE tree + is_ge | PE matmuls.
"""

from contextlib import ExitStack

import ml_dtypes
import numpy as np

import concourse.bass as bass
import concourse.mybir as mybir
from concourse.bass_utils import run_bass_kernel_spmd

B, C, H, W = 8, 21, 512, 512
N = H * W              # 262144 pixels per batch
P = 128                # SBUF partitions
S = 6                  # pixel slots per PE row
CW = C * S             # x cols per group (126), class-major
YW = 128               # y group width: 126 + 2 pad cols
HWC = (C + 1) * S      # h cols per group (132): 126 one-hot + 6 ones
NG = 342               # total groups per core (342*768 = 262656 >= N)
NPAD = NG * P * S      # padded pixel count
G_TILES = [10, 22, 44, 57, 57, 57, 57, 26, 12]
NT = len(G_TILES)
G_OFF = [sum(G_TILES[:i]) for i in range(NT)]
GMAX = max(G_TILES)
NEG = -65504.0
N_CORES = 8

_CACHED_NC = None


def build_nc():
    nc = bass.Bass()
    x = nc.declare_dram_parameter("x", [P, NG * CW], mybir.dt.float16, isOutput=False)
    y = nc.declare_dram_parameter("y", [P, NG * YW], mybir.dt.float8e4, isOutput=False)
    out = nc.declare_dram_parameter("out", [P, HWC], mybir.dt.float32, isOutput=True)

    mx = mybir.AluOpType.max

    with ExitStack() as ctx:
        xs = [
            ctx.enter_context(
                nc.sbuf_tensor(f"xsb{i}", [P, GMAX * CW], mybir.dt.float16)
            )
            for i in range(4)
        ]
        ys = [
            ctx.enter_context(
                nc.sbuf_tensor(f"ysb{i}", [P, GMAX * YW], mybir.dt.float8e4)
            )
            for i in range(3)
        ]
        hs = [
            ctx.enter_context(
                nc.sbuf_tensor(f"hsb{i}", [P, GMAX * HWC], mybir.dt.float16)
            )
            for i in range(3)
        ]
        tb1 = ctx.enter_context(nc.sbuf_tensor("tb1b", [P, GMAX * 42], mybir.dt.float16))
        tb2 = ctx.enter_context(nc.sbuf_tensor("tb2b", [P, GMAX * 42], mybir.dt.float16))
        t4 = ctx.enter_context(nc.sbuf_tensor("t4b", [P, GMAX * 24], mybir.dt.float16))
        t2 = ctx.enter_context(nc.sbuf_tensor("t2b", [P, GMAX * 12], mybir.dt.float16))
        m1s = [
            ctx.enter_context(nc.sbuf_tensor(f"m1b{i}", [P, GMAX * 6], mybir.dt.float16))
            for i in range(2)
        ]
        osb = ctx.enter_context(nc.sbuf_tensor("osb", [P, HWC], mybir.dt.float32))
        cm_psum = ctx.enter_context(nc.psum_tensor("cmps", [P, HWC], mybir.dt.float32))

        block = ctx.enter_context(nc.Block())
        sxs = [ctx.enter_context(nc.semaphore(f"sx{i}")) for i in range(4)]
        sys_ = [ctx.enter_context(nc.semaphore(f"sy{i}")) for i in range(3)]
        shd = ctx.enter_context(nc.semaphore("shd"))    # DVE isge(t) done, = t+1
        si = ctx.enter_context(nc.semaphore("si"))      # gpsimd ones init done
        sp = ctx.enter_context(nc.semaphore("sp"))      # PE tile matmuls done, = t+1
        sv2 = ctx.enter_context(nc.semaphore("sv2"))    # final psum copy done
        so = ctx.enter_context(nc.semaphore("so"))      # out DMA done

        def mview(buf, w, t):
            return (
                buf[:]
                .rearrange("p (g c) -> p g c", c=w)[:, 0 : G_TILES[t], :]
            )

        @block.sync
        def _(sync):
            for t in range(NT):
                if t >= 4:
                    sync.wait_ge(shd, t - 3)  # isge(t-4) freed x slot
                cols = G_TILES[t] * CW
                sync.dma_start(
                    out=xs[t % 4][:, 0:cols],
                    in_=x[:, G_OFF[t] * CW : G_OFF[t] * CW + cols],
                ).then_inc(sxs[t % 4], 16)
            sync.wait_ge(sv2, 1)
            sync.dma_start(out=out[:], in_=osb[:]).then_inc(so, 16)
            sync.wait_ge(so, 16)

        @block.scalar
        def _(scalar):
            # defer the first y-loads until isge(0) is done: x feeds the DVE
            # ramp and gets the full DMA bandwidth until then.
            scalar.wait_ge(shd, 1)
            for t in range(NT):
                if t >= 3:
                    scalar.wait_ge(sp, t - 2)  # matmul(t-3) freed y slot
                cols = G_TILES[t] * YW
                scalar.dma_start(
                    out=ys[t % 3][:, 0:cols],
                    in_=y[:, G_OFF[t] * YW : G_OFF[t] * YW + cols],
                ).then_inc(sys_[t % 3], 16)


        @block.gpsimd
        def _(gpsimd):
            h3v = [h[:].rearrange("p (g w) -> p g w", w=HWC) for h in hs]
            nc.gpsimd.memset(h3v[0][:, :, CW - S : HWC], 1.0)
            nc.gpsimd.memset(h3v[1][:, :, CW - S : HWC], 1.0)
            nc.gpsimd.memset(h3v[2][:, :, CW - S : HWC], 1.0)
            nc.gpsimd.memset(h3v[0][:, :, CW - S : CW], 0.0)
            nc.gpsimd.memset(h3v[1][:, :, CW - S : CW], 0.0)
            nc.gpsimd.memset(h3v[2][:, :, CW - S : CW], 0.0).then_inc(si, 1)

        @block.vector
        def _(vector):
            def isge(t):
                # h(t) one-hot: compare x against per-slot max, broadcast
                # over the 21 class blocks (inner step 1 keeps 2x mode)
                if t >= 3:
                    vector.wait_ge(sp, t - 2)  # matmul(t-3) freed h slot
                G = G_TILES[t]
                x4 = (
                    xs[t % 4][:]
                    .rearrange("p (g j s) -> p g j s", j=C, s=S)[:, 0:G, 0 : C - 1, :]
                )
                h4 = (
                    hs[t % 3][:]
                    .rearrange("p (g j s) -> p g j s", j=C + 1, s=S)
                )
                m1b = (
                    m1s[t % 2][:]
                    .rearrange("p (g s) -> p g s", s=S)[:, 0:G, :]
                    .unsqueeze(2)
                    .to_broadcast((P, G, C - 1, S))
                )
                nc.vector.tensor_tensor(
                    out=h4[:, 0:G, 0 : C - 1, :],
                    in0=x4,
                    in1=m1b,
                    op=mybir.AluOpType.is_ge,
                ).then_inc(shd, 1)

            for t in range(NT):
                vector.wait_ge(sxs[t % 4], 16 * (t // 4 + 1))
                x3 = mview(xs[t % 4], CW, t)
                tb1v = mview(tb1, 42, t)
                tb2v = mview(tb2, 42, t)
                t4v = mview(t4, 24, t)
                t2v = mview(t2, 12, t)
                m1v = mview(m1s[t % 2], 6, t)
                nc.vector.tensor_tensor(
                    out=tb1v, in0=x3[:, :, 0:42], in1=x3[:, :, 42:84], op=mx
                )
                nc.vector.tensor_tensor(
                    out=tb2v, in0=tb1v, in1=x3[:, :, 84:126], op=mx
                )
                if t >= 1:
                    # weave isge(t-1): independent of T2(t), overlaps its DRAIN
                    isge(t - 1)
                nc.vector.tensor_tensor(
                    out=t4v, in0=tb2v[:, :, 0:24], in1=tb2v[:, :, 18:42], op=mx
                )
                nc.vector.tensor_tensor(
                    out=t2v, in0=t4v[:, :, 0:12], in1=t4v[:, :, 12:24], op=mx
                )
                nc.vector.tensor_tensor(
                    out=m1v, in0=t2v[:, :, 0:6], in1=t2v[:, :, 6:12], op=mx
                )
            isge(NT - 1)
            vector.wait_ge(sp, NT)
            nc.vector.tensor_copy(osb[:], cm_psum[:]).then_inc(sv2, 1)

        @block.tensor
        def _(tensor):
            tensor.wait_ge(si, 1)
            for t in range(NT):
                tensor.wait_ge(sys_[t % 3], 16 * (t // 3 + 1))
                tensor.wait_ge(shd, t + 1)
                for g in range(G_TILES[t]):
                    mm = nc.tensor.matmul(
                        out=cm_psum[:],
                        lhsT=ys[t % 3][:, g * YW : (g + 1) * YW],
                        rhs=hs[t % 3][:, g * HWC : (g + 1) * HWC],
                        start=(t == 0 and g == 0),
                        stop=(t == NT - 1 and g == G_TILES[t] - 1),
                    )
                mm.then_inc(sp, 1)

    return nc


def _get_nc():
    global _CACHED_NC
    if _CACHED_NC is None:
        _CACHED_NC = build_nc()
    return _CACHED_NC


def make_in_maps(input, target):
    inp = np.asarray(input, dtype=np.float32)
    tgt = np.asarray(target, dtype=np.float32)
    in_maps = []
    for b in range(B):
        xb = inp[b].reshape(C, N).T  # [N, C]
        xq = np.full((NPAD, C), NEG, dtype=np.float16)
        xq[:N] = xb
        # pad pixels keep x = NEG everywhere -> h row all-ones but y rows are 0
        # class-major: [P, NG, C, S]
        x_dev = np.ascontiguousarray(
            xq.reshape(NG, S, P, C).transpose(2, 0, 3, 1)
        ).reshape(P, NG * CW)

        yb = tgt[b].reshape(C, N).T  # [N, C]
        yq = np.zeros((NPAD, C), dtype=np.float32)
        yq[:N] = yb
        y4 = yq.reshape(NG, S, P, C).transpose(2, 0, 3, 1)  # [P,NG,C,S]
        y_dev = np.zeros((P, NG, YW), dtype=ml_dtypes.float8_e4m3)
        y_dev[..., :CW] = y4.reshape(P, NG, CW).astype(ml_dtypes.float8_e4m3)
        in_maps.append({"x": x_dev, "y": y_dev.reshape(P, NG * YW)})
    return in_maps


def postprocess(outs):
    final = np.zeros((C, C), dtype=np.float64)
    for o in outs:
        o = np.asarray(o, dtype=np.float64)  # [128, 132]
        ov = o[:CW, :CW].reshape(C, S, C, S)
        cm = np.einsum("isjs->ij", ov)
        rsv = o[:CW, CW:HWC].reshape(C, S, S)
        rs = np.einsum("iss->i", rsv).reshape(C, 1)
        # device leaves class-block 20 zeroed; derive it from the row sum
        cm[:, C - 1] = rs[:, 0] - cm.sum(axis=1)
        final += cm / (rs + 1e-8)
    return (final / len(outs)).astype(np.float32)


def kernel(input, target):
    nc = _get_nc()
    in_maps = make_in_maps(input, target)
    res = run_bass_kernel_spmd(nc, in_maps, list(range(N_CORES)))
    return postprocess([r["out"] for r in res.results])
